# revision 1
# baseline (speedup 1.0000x reference)
"""Multi-head attention (B=2, L=2048, D=1024, H=16) on 8 trn2 NeuronCores.

Sharding: core c -> (batch b = c // 4, head group g = c % 4). Each core
computes 4 heads of one batch: Q/K/V projections restricted to its 256
projection columns, per-head attention, and a row-parallel slice of the
output projection. The 4 partial out-projections per batch are summed on
the host during unshard (bo and the V-bias term bv @ Wo are folded in
there too, since softmax rows sum to 1).

Per-core pipeline (all matmuls 128x128x512-shaped passes):
  - Q/K/V proj in bf16 (host-quantized x and W), accumulating fp32 PSUM.
  - qhT/khT kept transposed [head_dim, seq] in fp32r; bias added on DVE.
  - QK^T: two heads per pass via row packing (K=64 each at array rows
    0-63 / 64-127), St = [keys 128, 2 heads x 512 queries] PSUM.
  - exp on ScalarE -> bf16 P in SBUF.
  - AV + ones-denominator in bf16, col-packed pairs, accumulating over
    all 16 key blocks in PSUM.
  - normalize with reciprocal_approx_fast, out-proj in fp32r.
"""

import numpy as np
import ml_dtypes

B, L, D, H = 2, 2048, 1024, 16
HD = D // H          # 64
SCALE = HD ** -0.5
G = 4                # head groups (cores per batch)
HPG = H // G         # heads per group = 4
GC = HPG * HD        # projection cols per group = 256
N_CORES = 8
QC = 4               # query chunks of 512
KB = 16              # key blocks of 128
KT = D // 128        # contraction tiles = 8

_cache = {}


def _build():
    import concourse.mybir as mybir
    import concourse.tile as tile
    from concourse import bacc

    F32 = mybir.dt.float32
    F32R = mybir.dt.float32r
    BF16 = mybir.dt.bfloat16
    Exp = mybir.ActivationFunctionType.Exp

    nc = bacc.Bacc(None, target_bir_lowering=False, debug=False)

    with tile.TileContext(nc) as tc:
        with tc.tile_pool(name="dram", bufs=1, space="DRAM") as dram:
            def din(name, shape, dt):
                return dram.tile(shape, dt, kind="ExternalInput", name=name, uniquify=False)

            xq_d = din("xq", [QC, KT, 128, 512], BF16)   # q[b].T tiled (qc, kt)
            xk_d = din("xk", [QC, KT, 128, 512], BF16)   # k[b].T tiled (kc, kt)
            xv_d = din("xv", [QC, KT, 128, 512], BF16)
            wq_d = din("wq", [KT, 128, GC], BF16)
            wk_d = din("wk", [KT, 128, GC], BF16)
            wv_d = din("wv", [KT, 128, GC], BF16)
            wo_d = din("wo", [2, 128, D], F32R)          # Wo_g rows, fp32r-rounded
            bq_d = din("bq", [128, 2], F32)              # scaled q bias, per m-tile
            bk_d = din("bk", [128, 2], F32)
            out_d = dram.tile([8, QC, 128, 512], F32, kind="ExternalOutput",
                              name="out", uniquify=False)

            with (
                tc.tile_pool(name="const", bufs=1) as cpool,
                tc.tile_pool(name="xs", bufs=2) as xs_pool,
                tc.tile_pool(name="qk_sb", bufs=1) as qk_pool,
                tc.tile_pool(name="vh_sb", bufs=1) as vh_pool,
                tc.tile_pool(name="p_sb", bufs=3) as p_pool,
                tc.tile_pool(name="o_sb", bufs=3) as o_pool,
                tc.tile_pool(name="ps", bufs=2, space="PSUM") as ps_pool,
                tc.tile_pool(name="st_ps", bufs=2, space="PSUM") as st_pool,
                tc.tile_pool(name="acc_ps", bufs=1, space="PSUM") as acc_pool,
            ):
                # ---- constants / weights ----
                ones = cpool.tile([128, 64], BF16)
                nc.vector.memset(ones[:], 1.0)
                bq = cpool.tile([128, 2], F32)
                nc.sync.dma_start(bq[:], bq_d[:])
                bk = cpool.tile([128, 2], F32)
                nc.sync.dma_start(bk[:], bk_d[:])
                wq = cpool.tile([128, KT, GC], BF16)
                wk = cpool.tile([128, KT, GC], BF16)
                wv = cpool.tile([128, KT, GC], BF16)
                for kt in range(KT):
                    nc.sync.dma_start(wq[:, kt, :], wq_d[kt])
                    nc.sync.dma_start(wk[:, kt, :], wk_d[kt])
                    nc.sync.dma_start(wv[:, kt, :], wv_d[kt])
                wo = cpool.tile([128, 2, D], F32R)
                for p in range(2):
                    nc.sync.dma_start(wo[:, p, :], wo_d[p])

                # persistent activations
                qhT = [qk_pool.tile([128, L], F32R, name=f"qhT{p}") for p in range(2)]
                khT = [qk_pool.tile([128, L], F32R, name=f"khT{p}") for p in range(2)]
                vh = vh_pool.tile([128, KB, GC], BF16)
                ctxT = [qk_pool.tile([128, L], F32R, name=f"ctxT{p}") for p in range(2)]

                # ---- Q projection: qhT[p][:, qc] = (Wq^T x^T + bq) ----
                for qc in range(QC):
                    xq = xs_pool.tile([128, KT, 512], BF16, tag="x")
                    for kt in range(KT):
                        nc.sync.dma_start(xq[:, kt, :], xq_d[qc, kt])
                    for p in range(2):
                        acc = ps_pool.tile([128, 512], F32, tag="proj")
                        for kt in range(KT):
                            nc.tensor.matmul(
                                acc[:], wq[:, kt, p * 128:(p + 1) * 128], xq[:, kt, :],
                                start=(kt == 0), stop=(kt == KT - 1),
                            )
                        nc.vector.tensor_scalar_add(
                            qhT[p][:, qc * 512:(qc + 1) * 512], acc[:], bq[:, p:p + 1])

                # ---- K projection ----
                for kc in range(QC):
                    xk = xs_pool.tile([128, KT, 512], BF16, tag="x")
                    for kt in range(KT):
                        nc.sync.dma_start(xk[:, kt, :], xk_d[kc, kt])
                    for p in range(2):
                        acc = ps_pool.tile([128, 512], F32, tag="proj")
                        for kt in range(KT):
                            nc.tensor.matmul(
                                acc[:], wk[:, kt, p * 128:(p + 1) * 128], xk[:, kt, :],
                                start=(kt == 0), stop=(kt == KT - 1),
                            )
                        nc.vector.tensor_scalar_add(
                            khT[p][:, kc * 512:(kc + 1) * 512], acc[:], bk[:, p:p + 1])

                # ---- V projection: vh[kb] = [keys 128, 256] (no bias; folded on host) ----
                for kc in range(QC):
                    xv = xs_pool.tile([128, KT, 512], BF16, tag="x")
                    for kt in range(KT):
                        nc.sync.dma_start(xv[:, kt, :], xv_d[kc, kt])
                    for kbl in range(4):
                        kb = kc * 4 + kbl
                        acc = ps_pool.tile([128, 512], F32, tag="proj")
                        for kt in range(KT):
                            nc.tensor.matmul(
                                acc[:, 0:GC], xv[:, kt, kbl * 128:(kbl + 1) * 128],
                                wv[:, kt, :],
                                start=(kt == 0), stop=(kt == KT - 1),
                            )
                        nc.vector.tensor_copy(vh[:, kb, :], acc[:, 0:GC])

                # ---- attention ----
                for p in range(2):
                    for qc in range(QC):
                        ctx = acc_pool.tile([128, 512], F32, tag="ctx")
                        den = acc_pool.tile([128, 512], F32, tag="den")
                        q0 = qhT[p][0:64, qc * 512:(qc + 1) * 512]
                        q1 = qhT[p][64:128, qc * 512:(qc + 1) * 512]
                        for kb in range(KB):
                            St = st_pool.tile([128, 1024], F32, tag="St")
                            k_kb = khT[p][:, kb * 128:(kb + 1) * 128]
                            nc.tensor.matmul(St[:, 0:512], k_kb[0:64, :], q0,
                                             start=True, stop=True, tile_position=(0, 0))
                            nc.tensor.matmul(St[:, 512:1024], k_kb[64:128, :], q1,
                                             start=True, stop=True, tile_position=(64, 0))
                            P = p_pool.tile([128, 1024], BF16, tag="P")
                            nc.scalar.activation(P[:], St[:], Exp)

                            first, last = kb == 0, kb == KB - 1
                            v_kb = vh[:, kb, :]
                            nc.tensor.matmul(
                                ctx[0:64, :], v_kb[:, p * 128:p * 128 + 64], P[:, 0:512],
                                start=first, stop=last, tile_position=(0, 0))
                            nc.tensor.matmul(
                                ctx[64:128, :], v_kb[:, p * 128 + 64:p * 128 + 128],
                                P[:, 512:1024],
                                start=first, stop=last, tile_position=(0, 64))
                            nc.tensor.matmul(
                                den[0:64, :], ones[:, 0:64], P[:, 0:512],
                                start=first, stop=last, tile_position=(0, 0))
                            nc.tensor.matmul(
                                den[64:128, :], ones[:, 0:64], P[:, 512:1024],
                                start=first, stop=last, tile_position=(0, 64))

                        den_sb = o_pool.tile([128, 512], F32, tag="den_sb")
                        nc.vector.tensor_copy(den_sb[:], den[:])
                        recip = o_pool.tile([128, 512], F32, tag="recip")
                        nc.vector.reciprocal_approx_fast(recip[:], den_sb[:])
                        nc.vector.tensor_mul(
                            ctxT[p][:, qc * 512:(qc + 1) * 512], ctx[:], recip[:])

                # ---- out projection: out[et, qc] = sum_p Wo[p,et]^T @ ctxT[p] ----
                for qc in range(QC):
                    for et in range(8):
                        acc = ps_pool.tile([128, 512], F32, tag="proj")
                        for p in range(2):
                            nc.tensor.matmul(
                                acc[:], wo[:, p, et * 128:(et + 1) * 128],
                                ctxT[p][:, qc * 512:(qc + 1) * 512],
                                start=(p == 0), stop=(p == 1),
                            )
                        o_sb = o_pool.tile([128, 512], F32, tag="o_sb")
                        nc.vector.tensor_copy(o_sb[:], acc[:])
                        nc.sync.dma_start(out_d[et, qc], o_sb[:])

    nc.compile()
    return nc


def _round_fp32r(x):
    u = np.ascontiguousarray(x, dtype=np.float32).view(np.uint32).copy()
    u += 0x7FF + ((u >> 12) & 1)
    u &= np.uint32(0xFFFFF000)
    return u.view(np.float32)


def _tile_xT(x):
    """x (L, D) f32 -> x.T tiled [QC, KT, 128, 512] bf16 (contiguous tiles)."""
    xT = np.ascontiguousarray(x.T.astype(ml_dtypes.bfloat16))     # (D, L)
    t = xT.reshape(KT, 128, QC, 512).transpose(2, 0, 1, 3)        # (QC, KT, 128, 512)
    return np.ascontiguousarray(t)


def make_in_maps(q, k, v, Wq, bq, Wk, bk, Wv, bv, Wo, bo):
    in_maps = []
    host_const = []  # per-batch additive constant (bo + bv_g @ Wo_g summed over g)
    for c in range(N_CORES):
        b, g = divmod(c, G)
        cols = slice(g * GC, (g + 1) * GC)
        wq_g = (Wq[:, cols] * SCALE).astype(ml_dtypes.bfloat16)
        wk_g = Wk[:, cols].astype(ml_dtypes.bfloat16)
        wv_g = Wv[:, cols].astype(ml_dtypes.bfloat16)
        wo_g = _round_fp32r(Wo[cols, :])                           # (256, D)
        in_maps.append({
            "xq": _tile_xT(q[b]),
            "xk": _tile_xT(k[b]),
            "xv": _tile_xT(v[b]),
            "wq": np.ascontiguousarray(wq_g.reshape(KT, 128, GC)),
            "wk": np.ascontiguousarray(wk_g.reshape(KT, 128, GC)),
            "wv": np.ascontiguousarray(wv_g.reshape(KT, 128, GC)),
            "wo": np.ascontiguousarray(wo_g.reshape(2, 128, D)),
            "bq": np.ascontiguousarray((bq[cols] * SCALE).reshape(2, 128).T),
            "bk": np.ascontiguousarray(bk[cols].reshape(2, 128).T),
        })
    for b in range(B):
        host_const.append(bo + bv @ Wo)
    return in_maps, host_const


def assemble(results, host_const):
    """results: per-core dicts with 'out' [8, QC, 128, 512] = out_partial^T tiles."""
    out = np.zeros((B, L, D), np.float32)
    for c in range(N_CORES):
        b = c // G
        t = results[c]["out"]                                     # (8, QC, 128, 512)
        full = t.transpose(0, 2, 1, 3).reshape(D, L)              # out_partial^T
        out[b] += full.T
    for b in range(B):
        out[b] += host_const[b].astype(np.float32)
    return out


def kernel(q, k, v, Wq, bq, Wk, bk, Wv, bv, Wo, bo):
    from concourse.bass_utils import run_bass_kernel_spmd

    if "nc" not in _cache:
        _cache["nc"] = _build()
    nc = _cache["nc"]
    args = (q, k, v, Wq, bq, Wk, bk, Wv, bv, Wo, bo)
    args = tuple(np.asarray(a, dtype=np.float32) for a in args)
    in_maps, host_const = make_in_maps(*args)
    res = run_bass_kernel_spmd(nc, in_maps, list(range(N_CORES)))
    return assemble(res.results, host_const)


# revision 3
# speedup vs baseline: 284.0927x; 284.0927x over previous
"""Multi-head attention (B=2, L=2048, D=1024, H=16) on 8 trn2 NeuronCores.

Sharding: core c -> (batch b = c // 4, head group g = c % 4). Each core
computes 4 heads of one batch: Q/K/V projections restricted to its 256
projection columns, per-head attention, and a row-parallel slice of the
output projection. The 4 partial out-projections per batch are summed on
the host during unshard (bo and the V-bias term bv @ Wo are folded in
there too, since softmax rows sum to 1).

Per-core pipeline (all matmuls 128x128x512-shaped passes):
  - Q/K/V proj in bf16 (host-quantized x and W), accumulating fp32 PSUM.
  - qhT/khT kept transposed [head_dim, seq] in fp32r; bias added on DVE.
  - QK^T: two heads per pass via row packing (K=64 each at array rows
    0-63 / 64-127), St = [keys 128, 2 heads x 512 queries] PSUM.
  - exp on ScalarE -> bf16 P in SBUF.
  - AV + ones-denominator in bf16, col-packed pairs, accumulating over
    all 16 key blocks in PSUM.
  - normalize with reciprocal_approx_fast; out-proj in fp32r interleaved
    per query chunk so PE fills the ACT-bound attention window.
"""

import numpy as np
import ml_dtypes

B, L, D, H = 2, 2048, 1024, 16
HD = D // H          # 64
SCALE = HD ** -0.5
G = 4                # head groups (cores per batch)
HPG = H // G         # heads per group = 4
GC = HPG * HD        # projection cols per group = 256
N_CORES = 8
QC = 4               # query chunks of 512
KB = 16              # key blocks of 128
KT = D // 128        # contraction tiles = 8

_cache = {}


def _emit(nc, tc, tiles, mybir):
    F32 = mybir.dt.float32
    F32R = mybir.dt.float32r
    BF16 = mybir.dt.bfloat16
    Exp = mybir.ActivationFunctionType.Exp
    (xq_d, xk_d, xv_d, wq_d, wk_d, wv_d, wo_d, bq_d, bk_d, out_d,
     cpool, xs_pool, qk_pool, vh_pool, p_pool, o_pool, ps_pool, st_pool, acc_pool) = tiles

    # ---- constants / weights ----
    ones = cpool.tile([128, 64], BF16, tag="ones")
    nc.vector.memset(ones[:], 1.0)
    bq = cpool.tile([128, 2], F32, tag="bq")
    nc.sync.dma_start(bq[:], bq_d[:])
    bk = cpool.tile([128, 2], F32, tag="bk")
    nc.sync.dma_start(bk[:], bk_d[:])
    wq = cpool.tile([128, KT, GC], BF16, tag="wq")
    wk = cpool.tile([128, KT, GC], BF16, tag="wk")
    wv = cpool.tile([128, KT, GC], BF16, tag="wv")
    for kt in range(KT):
        nc.sync.dma_start(wq[:, kt, :], wq_d[kt])
        nc.sync.dma_start(wk[:, kt, :], wk_d[kt])
        nc.sync.dma_start(wv[:, kt, :], wv_d[kt])
    wo = cpool.tile([128, 2, D], F32R, tag="wo")
    for p in range(2):
        nc.sync.dma_start(wo[:, p, :], wo_d[p])

    # persistent activations
    qhT = [qk_pool.tile([128, L], F32R, tag=f"qhT{p}", name=f"qhT{p}") for p in range(2)]
    khT = [qk_pool.tile([128, L], F32R, tag=f"khT{p}", name=f"khT{p}") for p in range(2)]
    vh = vh_pool.tile([128, KB, GC], BF16, tag="vh")
    ctxT = [qk_pool.tile([128, L], F32R, tag=f"ctxT{p}", name=f"ctxT{p}") for p in range(2)]

    # ---- Q projection: qhT[p][:, qc] = (Wq^T x^T + bq) ----
    for qc in range(QC):
        xq = xs_pool.tile([128, KT, 512], BF16, tag="x")
        for kt in range(KT):
            nc.sync.dma_start(xq[:, kt, :], xq_d[qc, kt])
        for p in range(2):
            acc = ps_pool.tile([128, 512], F32, tag="proj")
            for kt in range(KT):
                nc.tensor.matmul(
                    acc[:], wq[:, kt, p * 128:(p + 1) * 128], xq[:, kt, :],
                    start=(kt == 0), stop=(kt == KT - 1),
                )
            nc.vector.tensor_scalar_add(
                qhT[p][:, qc * 512:(qc + 1) * 512], acc[:], bq[:, p:p + 1])

    # ---- K projection ----
    for kc in range(QC):
        xk = xs_pool.tile([128, KT, 512], BF16, tag="x")
        for kt in range(KT):
            nc.sync.dma_start(xk[:, kt, :], xk_d[kc, kt])
        for p in range(2):
            acc = ps_pool.tile([128, 512], F32, tag="proj")
            for kt in range(KT):
                nc.tensor.matmul(
                    acc[:], wk[:, kt, p * 128:(p + 1) * 128], xk[:, kt, :],
                    start=(kt == 0), stop=(kt == KT - 1),
                )
            nc.vector.tensor_scalar_add(
                khT[p][:, kc * 512:(kc + 1) * 512], acc[:], bk[:, p:p + 1])

    # ---- V projection: vh[kb] = [keys 128, 256] (no bias; folded on host) ----
    for kc in range(QC):
        xv = xs_pool.tile([128, KT, 512], BF16, tag="x")
        for kt in range(KT):
            nc.sync.dma_start(xv[:, kt, :], xv_d[kc, kt])
        for kbl in range(4):
            kb = kc * 4 + kbl
            acc = ps_pool.tile([128, 512], F32, tag="proj")
            for kt in range(KT):
                nc.tensor.matmul(
                    acc[:, 0:GC], xv[:, kt, kbl * 128:(kbl + 1) * 128],
                    wv[:, kt, :],
                    start=(kt == 0), stop=(kt == KT - 1),
                )
            nc.vector.tensor_copy(vh[:, kb, :], acc[:, 0:GC])

    # ---- attention + interleaved out-projection ----
    for qc in range(QC):
        for p in range(2):
            ctx = acc_pool.tile([128, 512], F32, tag="ctx")
            den = acc_pool.tile([128, 512], F32, tag="den")
            q0 = qhT[p][0:64, qc * 512:(qc + 1) * 512]
            q1 = qhT[p][64:128, qc * 512:(qc + 1) * 512]
            for kb in range(KB):
                St = st_pool.tile([128, 1024], F32, tag="St")
                k_kb = khT[p][:, kb * 128:(kb + 1) * 128]
                nc.tensor.matmul(St[:, 0:512], k_kb[0:64, :], q0,
                                 start=True, stop=True, tile_position=(0, 0))
                nc.tensor.matmul(St[:, 512:1024], k_kb[64:128, :], q1,
                                 start=True, stop=True, tile_position=(64, 0))
                P = p_pool.tile([128, 1024], BF16, tag="P")
                nc.scalar.activation(P[:], St[:], Exp)

                first, last = kb == 0, kb == KB - 1
                v_kb = vh[:, kb, :]
                nc.tensor.matmul(
                    ctx[0:64, :], v_kb[:, p * 128:p * 128 + 64], P[:, 0:512],
                    start=first, stop=last, tile_position=(0, 0))
                nc.tensor.matmul(
                    ctx[64:128, :], v_kb[:, p * 128 + 64:p * 128 + 128],
                    P[:, 512:1024],
                    start=first, stop=last, tile_position=(0, 64))
                nc.tensor.matmul(
                    den[0:64, :], ones[:, 0:64], P[:, 0:512],
                    start=first, stop=last, tile_position=(0, 0))
                nc.tensor.matmul(
                    den[64:128, :], ones[:, 0:64], P[:, 512:1024],
                    start=first, stop=last, tile_position=(0, 64))

            den_sb = o_pool.tile([128, 512], F32, tag="den_sb")
            nc.vector.tensor_copy(den_sb[:], den[:])
            recip = o_pool.tile([128, 512], F32, tag="recip")
            nc.vector.reciprocal_approx_fast(recip[:], den_sb[:])
            nc.vector.tensor_mul(
                ctxT[p][:, qc * 512:(qc + 1) * 512], ctx[:], recip[:])

        # out-proj for this query chunk (both pairs ready)
        for et in range(8):
            acc = ps_pool.tile([128, 512], F32, tag="proj")
            for p in range(2):
                nc.tensor.matmul(
                    acc[:], wo[:, p, et * 128:(et + 1) * 128],
                    ctxT[p][:, qc * 512:(qc + 1) * 512],
                    start=(p == 0), stop=(p == 1),
                )
            o_sb = o_pool.tile([128, 512], F32, tag="o_sb")
            nc.vector.tensor_copy(o_sb[:], acc[:])
            nc.sync.dma_start(out_d[et, qc], o_sb[:])


def _build(loop_k=None):
    import concourse.mybir as mybir
    import concourse.tile as tile
    from concourse import bacc

    F32 = mybir.dt.float32
    F32R = mybir.dt.float32r
    BF16 = mybir.dt.bfloat16

    nc = bacc.Bacc(None, target_bir_lowering=False, debug=False)

    with tile.TileContext(nc) as tc:
        with tc.tile_pool(name="dram", bufs=1, space="DRAM") as dram:
            def din(name, shape, dt):
                return dram.tile(shape, dt, kind="ExternalInput", name=name, uniquify=False)

            dram_tiles = (
                din("xq", [QC, KT, 128, 512], BF16),
                din("xk", [QC, KT, 128, 512], BF16),
                din("xv", [QC, KT, 128, 512], BF16),
                din("wq", [KT, 128, GC], BF16),
                din("wk", [KT, 128, GC], BF16),
                din("wv", [KT, 128, GC], BF16),
                din("wo", [2, 128, D], F32R),
                din("bq", [128, 2], F32),
                din("bk", [128, 2], F32),
                dram.tile([8, QC, 128, 512], F32, kind="ExternalOutput",
                          name="out", uniquify=False),
            )

            with (
                tc.tile_pool(name="const", bufs=1) as cpool,
                tc.tile_pool(name="xs", bufs=2) as xs_pool,
                tc.tile_pool(name="qk_sb", bufs=1) as qk_pool,
                tc.tile_pool(name="vh_sb", bufs=1) as vh_pool,
                tc.tile_pool(name="p_sb", bufs=3) as p_pool,
                tc.tile_pool(name="o_sb", bufs=3) as o_pool,
                tc.tile_pool(name="ps", bufs=2, space="PSUM") as ps_pool,
                tc.tile_pool(name="st_ps", bufs=2, space="PSUM") as st_pool,
                tc.tile_pool(name="acc_ps", bufs=1, space="PSUM") as acc_pool,
            ):
                tiles = dram_tiles + (cpool, xs_pool, qk_pool, vh_pool, p_pool,
                                      o_pool, ps_pool, st_pool, acc_pool)
                if loop_k is None:
                    _emit(nc, tc, tiles, mybir)
                else:
                    with tc.For_i(0, loop_k, 1):
                        _emit(nc, tc, tiles, mybir)

    nc.compile()
    return nc


def _round_fp32r(x):
    u = np.ascontiguousarray(x, dtype=np.float32).view(np.uint32).copy()
    u += 0x7FF + ((u >> 12) & 1)
    u &= np.uint32(0xFFFFF000)
    return u.view(np.float32)


def _tile_xT(x):
    """x (L, D) f32 -> x.T tiled [QC, KT, 128, 512] bf16 (contiguous tiles)."""
    xT = np.ascontiguousarray(x.T.astype(ml_dtypes.bfloat16))     # (D, L)
    t = xT.reshape(KT, 128, QC, 512).transpose(2, 0, 1, 3)        # (QC, KT, 128, 512)
    return np.ascontiguousarray(t)


def make_in_maps(q, k, v, Wq, bq, Wk, bk, Wv, bv, Wo, bo):
    in_maps = []
    for c in range(N_CORES):
        b, g = divmod(c, G)
        cols = slice(g * GC, (g + 1) * GC)
        wq_g = (Wq[:, cols] * SCALE).astype(ml_dtypes.bfloat16)
        wk_g = Wk[:, cols].astype(ml_dtypes.bfloat16)
        wv_g = Wv[:, cols].astype(ml_dtypes.bfloat16)
        wo_g = _round_fp32r(Wo[cols, :])                           # (256, D)
        in_maps.append({
            "xq": _tile_xT(q[b]),
            "xk": _tile_xT(k[b]),
            "xv": _tile_xT(v[b]),
            "wq": np.ascontiguousarray(wq_g.reshape(KT, 128, GC)),
            "wk": np.ascontiguousarray(wk_g.reshape(KT, 128, GC)),
            "wv": np.ascontiguousarray(wv_g.reshape(KT, 128, GC)),
            "wo": np.ascontiguousarray(wo_g.reshape(2, 128, D)),
            "bq": np.ascontiguousarray((bq[cols] * SCALE).reshape(2, 128).T),
            "bk": np.ascontiguousarray(bk[cols].reshape(2, 128).T),
        })
    host_const = bo + bv @ Wo
    return in_maps, host_const


def assemble(results, host_const):
    """results: per-core dicts with 'out' [8, QC, 128, 512] = out_partial^T tiles."""
    out = np.zeros((B, L, D), np.float32)
    for c in range(N_CORES):
        b = c // G
        t = results[c]["out"]                                     # (8, QC, 128, 512)
        full = t.transpose(0, 2, 1, 3).reshape(D, L)              # out_partial^T
        out[b] += full.T
    out += host_const.astype(np.float32)
    return out


def kernel(q, k, v, Wq, bq, Wk, bk, Wv, bv, Wo, bo):
    from concourse.bass_utils import run_bass_kernel_spmd

    if "nc" not in _cache:
        _cache["nc"] = _build()
    nc = _cache["nc"]
    args = (q, k, v, Wq, bq, Wk, bk, Wv, bv, Wo, bo)
    args = tuple(np.asarray(a, dtype=np.float32) for a in args)
    in_maps, host_const = make_in_maps(*args)
    res = run_bass_kernel_spmd(nc, in_maps, list(range(N_CORES)))
    return assemble(res.results, host_const)


# revision 4
# speedup vs baseline: 382.2434x; 1.3455x over previous
"""Multi-head attention (B=2, L=2048, D=1024, H=16) on 8 trn2 NeuronCores.

Sharding: core c -> (batch b = c // 4, head group g = c % 4). Each core
computes 4 heads of one batch: Q/K/V projections restricted to its 256
projection columns, per-head attention, and a row-parallel slice of the
output projection. The 4 partial out-projections per batch are summed on
the host during unshard (bo and the V-bias term bv @ Wo are folded in
there too, since softmax rows sum to 1).

Per-core pipeline (all matmuls 128x128x512-shaped passes):
  - Q/K/V proj in bf16 (host-quantized x and W), accumulating fp32 PSUM.
  - qhT/khT kept transposed [head_dim, seq] in fp32r; bias added on DVE.
  - QK^T: two heads per pass via row packing (K=64 each at array rows
    0-63 / 64-127), St = [keys 128, 2 heads x 512 queries] PSUM.
  - exp on ScalarE -> bf16 P in SBUF.
  - AV + ones-denominator in bf16, col-packed pairs, accumulating over
    all 16 key blocks in PSUM.
  - normalize with reciprocal_approx_fast; out-proj in fp32r interleaved
    per query chunk so PE fills the ACT-bound attention window.
"""

import numpy as np
import ml_dtypes

B, L, D, H = 2, 2048, 1024, 16
HD = D // H          # 64
SCALE = HD ** -0.5
G = 4                # head groups (cores per batch)
HPG = H // G         # heads per group = 4
GC = HPG * HD        # projection cols per group = 256
N_CORES = 8
QC = 4               # query chunks of 512
KB = 16              # key blocks of 128
KT = D // 128        # contraction tiles = 8

_cache = {}


def _emit(nc, tc, tiles, mybir):
    F32 = mybir.dt.float32
    F32R = mybir.dt.float32r
    BF16 = mybir.dt.bfloat16
    Exp = mybir.ActivationFunctionType.Exp
    (xq_d, xk_d, xv_d, wq_d, wk_d, wv_d, wo_d, bq_d, bk_d, out_d,
     cpool, xs_pool, qk_pool, vh_pool, p_pool, o_pool, ps_pool, st_pool, acc_pool) = tiles

    # ---- constants / weights ----
    ones = cpool.tile([128, 64], BF16, tag="ones")
    nc.vector.memset(ones[:], 1.0)
    bq = cpool.tile([128, 2], F32, tag="bq")
    nc.sync.dma_start(bq[:], bq_d[:])
    bk = cpool.tile([128, 2], F32, tag="bk")
    nc.sync.dma_start(bk[:], bk_d[:])
    wq = cpool.tile([128, KT, GC], BF16, tag="wq")
    wk = cpool.tile([128, KT, GC], BF16, tag="wk")
    wv = cpool.tile([128, KT, GC], BF16, tag="wv")
    for kt in range(KT):
        nc.sync.dma_start(wq[:, kt, :], wq_d[kt])
        nc.sync.dma_start(wk[:, kt, :], wk_d[kt])
        nc.sync.dma_start(wv[:, kt, :], wv_d[kt])
    wo = cpool.tile([128, 2, D], F32R, tag="wo")
    for p in range(2):
        nc.sync.dma_start(wo[:, p, :], wo_d[p])

    # persistent activations
    qhT = [qk_pool.tile([128, L], F32R, tag=f"qhT{p}", name=f"qhT{p}") for p in range(2)]
    khT = [qk_pool.tile([128, L], F32R, tag=f"khT{p}", name=f"khT{p}") for p in range(2)]
    vh = vh_pool.tile([128, KB, GC], BF16, tag="vh")
    ctxT = [qk_pool.tile([128, L], F32R, tag=f"ctxT{p}", name=f"ctxT{p}") for p in range(2)]

    # ---- Q projection: qhT[p][:, qc] = (Wq^T x^T + bq) ----
    for qc in range(QC):
        xq = xs_pool.tile([128, KT, 512], BF16, tag="x")
        for kt in range(KT):
            nc.sync.dma_start(xq[:, kt, :], xq_d[qc, kt])
        for p in range(2):
            acc = ps_pool.tile([128, 512], F32, tag="proj")
            for kt in range(KT):
                nc.tensor.matmul(
                    acc[:], wq[:, kt, p * 128:(p + 1) * 128], xq[:, kt, :],
                    start=(kt == 0), stop=(kt == KT - 1),
                )
            nc.vector.tensor_scalar_add(
                qhT[p][:, qc * 512:(qc + 1) * 512], acc[:], bq[:, p:p + 1])

    # ---- K projection ----
    for kc in range(QC):
        xk = xs_pool.tile([128, KT, 512], BF16, tag="x")
        for kt in range(KT):
            nc.sync.dma_start(xk[:, kt, :], xk_d[kc, kt])
        for p in range(2):
            acc = ps_pool.tile([128, 512], F32, tag="proj")
            for kt in range(KT):
                nc.tensor.matmul(
                    acc[:], wk[:, kt, p * 128:(p + 1) * 128], xk[:, kt, :],
                    start=(kt == 0), stop=(kt == KT - 1),
                )
            nc.vector.tensor_scalar_add(
                khT[p][:, kc * 512:(kc + 1) * 512], acc[:], bk[:, p:p + 1])

    # ---- V projection: vh[kb] = [keys 128, 256] (no bias; folded on host) ----
    for kc in range(QC):
        xv = xs_pool.tile([128, KT, 512], BF16, tag="x")
        for kt in range(KT):
            nc.sync.dma_start(xv[:, kt, :], xv_d[kc, kt])
        for kbl in range(4):
            kb = kc * 4 + kbl
            acc = ps_pool.tile([128, 512], F32, tag="proj")
            for kt in range(KT):
                nc.tensor.matmul(
                    acc[:, 0:GC], xv[:, kt, kbl * 128:(kbl + 1) * 128],
                    wv[:, kt, :],
                    start=(kt == 0), stop=(kt == KT - 1),
                )
            nc.vector.tensor_copy(vh[:, kb, :], acc[:, 0:GC])

    # ---- attention + interleaved out-projection ----
    for qc in range(QC):
        for p in range(2):
            ctx = acc_pool.tile([128, 512], F32, tag="ctx")
            den = acc_pool.tile([128, 512], F32, tag="den")
            q0 = qhT[p][0:64, qc * 512:(qc + 1) * 512]
            q1 = qhT[p][64:128, qc * 512:(qc + 1) * 512]

            def qk(kb):
                St = st_pool.tile([128, 1024], F32, tag="St", name="St")
                k_kb = khT[p][:, kb * 128:(kb + 1) * 128]
                nc.tensor.matmul(St[:, 0:512], k_kb[0:64, :], q0,
                                 start=True, stop=True, tile_position=(0, 0))
                nc.tensor.matmul(St[:, 512:1024], k_kb[64:128, :], q1,
                                 start=True, stop=True, tile_position=(64, 0))
                return St

            # software pipeline: QK one key-block ahead so PE never waits on exp
            St_cur = qk(0)
            for kb in range(KB):
                P = p_pool.tile([128, 1024], BF16, tag="P")
                nc.scalar.activation(P[:], St_cur[:], Exp)
                if kb + 1 < KB:
                    St_cur = qk(kb + 1)

                first, last = kb == 0, kb == KB - 1
                v_kb = vh[:, kb, :]
                nc.tensor.matmul(
                    ctx[0:64, :], v_kb[:, p * 128:p * 128 + 64], P[:, 0:512],
                    start=first, stop=last, tile_position=(0, 0))
                nc.tensor.matmul(
                    ctx[64:128, :], v_kb[:, p * 128 + 64:p * 128 + 128],
                    P[:, 512:1024],
                    start=first, stop=last, tile_position=(0, 64))
                nc.tensor.matmul(
                    den[0:64, :], ones[:, 0:64], P[:, 0:512],
                    start=first, stop=last, tile_position=(0, 0))
                nc.tensor.matmul(
                    den[64:128, :], ones[:, 0:64], P[:, 512:1024],
                    start=first, stop=last, tile_position=(0, 64))

            den_sb = o_pool.tile([128, 512], F32, tag="den_sb")
            nc.vector.tensor_copy(den_sb[:], den[:])
            recip = o_pool.tile([128, 512], F32, tag="recip")
            nc.vector.reciprocal_approx_fast(recip[:], den_sb[:])
            nc.vector.tensor_mul(
                ctxT[p][:, qc * 512:(qc + 1) * 512], ctx[:], recip[:])

        # out-proj for this query chunk (both pairs ready)
        for et in range(8):
            acc = ps_pool.tile([128, 512], F32, tag="proj")
            for p in range(2):
                nc.tensor.matmul(
                    acc[:], wo[:, p, et * 128:(et + 1) * 128],
                    ctxT[p][:, qc * 512:(qc + 1) * 512],
                    start=(p == 0), stop=(p == 1),
                )
            o_sb = o_pool.tile([128, 512], F32, tag="o_sb")
            nc.vector.tensor_copy(o_sb[:], acc[:])
            nc.sync.dma_start(out_d[et, qc], o_sb[:])


def _build(loop_k=None):
    import concourse.mybir as mybir
    import concourse.tile as tile
    from concourse import bacc

    F32 = mybir.dt.float32
    F32R = mybir.dt.float32r
    BF16 = mybir.dt.bfloat16

    nc = bacc.Bacc(None, target_bir_lowering=False, debug=False)

    with tile.TileContext(nc) as tc:
        with tc.tile_pool(name="dram", bufs=1, space="DRAM") as dram:
            def din(name, shape, dt):
                return dram.tile(shape, dt, kind="ExternalInput", name=name, uniquify=False)

            dram_tiles = (
                din("xq", [QC, KT, 128, 512], BF16),
                din("xk", [QC, KT, 128, 512], BF16),
                din("xv", [QC, KT, 128, 512], BF16),
                din("wq", [KT, 128, GC], BF16),
                din("wk", [KT, 128, GC], BF16),
                din("wv", [KT, 128, GC], BF16),
                din("wo", [2, 128, D], F32R),
                din("bq", [128, 2], F32),
                din("bk", [128, 2], F32),
                dram.tile([8, QC, 128, 512], F32, kind="ExternalOutput",
                          name="out", uniquify=False),
            )

            with (
                tc.tile_pool(name="const", bufs=1) as cpool,
                tc.tile_pool(name="xs", bufs=2) as xs_pool,
                tc.tile_pool(name="qk_sb", bufs=1) as qk_pool,
                tc.tile_pool(name="vh_sb", bufs=1) as vh_pool,
                tc.tile_pool(name="p_sb", bufs=3) as p_pool,
                tc.tile_pool(name="o_sb", bufs=3) as o_pool,
                tc.tile_pool(name="ps", bufs=2, space="PSUM") as ps_pool,
                tc.tile_pool(name="st_ps", bufs=2, space="PSUM") as st_pool,
                tc.tile_pool(name="acc_ps", bufs=1, space="PSUM") as acc_pool,
            ):
                tiles = dram_tiles + (cpool, xs_pool, qk_pool, vh_pool, p_pool,
                                      o_pool, ps_pool, st_pool, acc_pool)
                if loop_k is None:
                    _emit(nc, tc, tiles, mybir)
                else:
                    with tc.For_i(0, loop_k, 1):
                        _emit(nc, tc, tiles, mybir)

    nc.compile()
    return nc


def _round_fp32r(x):
    u = np.ascontiguousarray(x, dtype=np.float32).view(np.uint32).copy()
    u += 0x7FF + ((u >> 12) & 1)
    u &= np.uint32(0xFFFFF000)
    return u.view(np.float32)


def _tile_xT(x):
    """x (L, D) f32 -> x.T tiled [QC, KT, 128, 512] bf16 (contiguous tiles)."""
    xT = np.ascontiguousarray(x.T.astype(ml_dtypes.bfloat16))     # (D, L)
    t = xT.reshape(KT, 128, QC, 512).transpose(2, 0, 1, 3)        # (QC, KT, 128, 512)
    return np.ascontiguousarray(t)


def make_in_maps(q, k, v, Wq, bq, Wk, bk, Wv, bv, Wo, bo):
    in_maps = []
    for c in range(N_CORES):
        b, g = divmod(c, G)
        cols = slice(g * GC, (g + 1) * GC)
        wq_g = (Wq[:, cols] * SCALE).astype(ml_dtypes.bfloat16)
        wk_g = Wk[:, cols].astype(ml_dtypes.bfloat16)
        wv_g = Wv[:, cols].astype(ml_dtypes.bfloat16)
        wo_g = _round_fp32r(Wo[cols, :])                           # (256, D)
        in_maps.append({
            "xq": _tile_xT(q[b]),
            "xk": _tile_xT(k[b]),
            "xv": _tile_xT(v[b]),
            "wq": np.ascontiguousarray(wq_g.reshape(KT, 128, GC)),
            "wk": np.ascontiguousarray(wk_g.reshape(KT, 128, GC)),
            "wv": np.ascontiguousarray(wv_g.reshape(KT, 128, GC)),
            "wo": np.ascontiguousarray(wo_g.reshape(2, 128, D)),
            "bq": np.ascontiguousarray((bq[cols] * SCALE).reshape(2, 128).T),
            "bk": np.ascontiguousarray(bk[cols].reshape(2, 128).T),
        })
    host_const = bo + bv @ Wo
    return in_maps, host_const


def assemble(results, host_const):
    """results: per-core dicts with 'out' [8, QC, 128, 512] = out_partial^T tiles."""
    out = np.zeros((B, L, D), np.float32)
    for c in range(N_CORES):
        b = c // G
        t = results[c]["out"]                                     # (8, QC, 128, 512)
        full = t.transpose(0, 2, 1, 3).reshape(D, L)              # out_partial^T
        out[b] += full.T
    out += host_const.astype(np.float32)
    return out


def kernel(q, k, v, Wq, bq, Wk, bk, Wv, bv, Wo, bo):
    from concourse.bass_utils import run_bass_kernel_spmd

    if "nc" not in _cache:
        _cache["nc"] = _build()
    nc = _cache["nc"]
    args = (q, k, v, Wq, bq, Wk, bk, Wv, bv, Wo, bo)
    args = tuple(np.asarray(a, dtype=np.float32) for a in args)
    in_maps, host_const = make_in_maps(*args)
    res = run_bass_kernel_spmd(nc, in_maps, list(range(N_CORES)))
    return assemble(res.results, host_const)
